# revision 13
# baseline (speedup 1.0000x reference)
"""Sparse 3D conv (gather -> per-offset matmul -> accumulate) on 8 TRN2 NeuronCores.

Strategy (data-parallel over output voxels, per the sharding hint):
  - Shard the N=200000 output voxels across 8 cores (25000 each, padded to
    25088 = 49 tiles x 512 voxels).
  - Host-side sharding prep builds each core's operand stream in the layout
    the PE consumes directly: for every tile, a [128, 7, 512] bf16 block
    whose partition rows are (k%4)*32+ch for kernel-offset group g=k//4
    (27 offsets padded to 28 = 7 groups of 4) and whose columns are the 512
    voxels, with masked/padded entries zeroed.
  - Replicate the small [128, 7, 64] grouped weight stack to every core.
  - Device per tile: one HWDGE stream DMA (898KB), 7 PE matmuls accumulating
    out^T[64, 512] f32 over the groups in PSUM, copy to SBUF as fp16, DMA to
    a per-core out^T[64, 25088] fp16 buffer. Memory-bound: ~48MB/core at
    line rate.
  - Host transposes/crops/concats per-core outputs to the full [200000, 64].
"""
import numpy as np
import ml_dtypes
from contextlib import ExitStack

import concourse.bass as bass
import concourse.bacc as bacc
import concourse.mybir as mybir
import concourse.tile as tile
from concourse.bass_utils import run_bass_kernel_spmd

N = 200000
K = 27
CIN = 32
COUT = 64
NCORES = 8
PERCORE = N // NCORES          # 25000
VTILE = 512
NTILES = (PERCORE + VTILE - 1) // VTILE   # 49
NPAD = NTILES * VTILE          # 25088
NGRP = 7                       # 28 k-slots (27 real + 1 zero) in groups of 4

_NC_CACHE = None


def _build_kernel():
    nc = bacc.Bacc("TRN2", target_bir_lowering=False)
    gts = nc.dram_tensor("gts", [NTILES, 128, 6 * VTILE], mybir.dt.bfloat16,
                         kind="ExternalInput")
    gts6 = nc.dram_tensor("gts6", [NTILES // 7, 96, 7 * VTILE],
                          mybir.dt.bfloat16, kind="ExternalInput")
    wst = nc.dram_tensor("wst", [128, NGRP * COUT], mybir.dt.bfloat16,
                         kind="ExternalInput")
    outT = nc.dram_tensor("outT", [COUT, NPAD], mybir.dt.float16,
                          kind="ExternalOutput")

    with tile.TileContext(nc) as tc, ExitStack() as ctx:
        const = ctx.enter_context(tc.tile_pool(name="const", bufs=1))
        sbg = ctx.enter_context(tc.tile_pool(name="sbg", bufs=6))
        sbg6 = ctx.enter_context(tc.tile_pool(name="sbg6", bufs=2))
        sbo = ctx.enter_context(tc.tile_pool(name="sbo", bufs=4))
        opsum = ctx.enter_context(tc.tile_pool(name="opsum", bufs=4, space="PSUM"))

        w_sb = const.tile([128, NGRP * COUT], mybir.dt.bfloat16, name="w_sb")
        nc.sync.dma_start(w_sb[:], wst[:])

        gt6 = None
        for t in range(NTILES):
            if t % 7 == 0:
                gt6 = sbg6.tile([96, 7 * VTILE], mybir.dt.bfloat16,
                                name="gt6", tag="gt6")
                nc.sync.dma_start(gt6[:], gts6[t // 7])
            gt = sbg.tile([128, 6 * VTILE], mybir.dt.bfloat16, name="gt",
                          tag="gt")
            nc.sync.dma_start(gt[:], gts[t])

            out_p = opsum.tile([COUT, VTILE], mybir.dt.float32, name="out_p",
                               tag="op")
            for g in range(6):
                nc.tensor.matmul(
                    out_p[:],
                    lhsT=w_sb[:, g * COUT:(g + 1) * COUT],
                    rhs=gt[:, g * VTILE:(g + 1) * VTILE],
                    start=(g == 0),
                    stop=False,
                )
            nc.tensor.matmul(
                out_p[:],
                lhsT=w_sb[0:96, 6 * COUT:7 * COUT],
                rhs=gt6[:, (t % 7) * VTILE:(t % 7 + 1) * VTILE],
                start=False,
                stop=True,
            )
            out_sb = sbo.tile([COUT, VTILE], mybir.dt.float16, name="out_sb",
                              tag="ob")
            if t % 2 == 0:
                nc.vector.tensor_copy(out_sb[:], out_p[:])
            else:
                nc.scalar.copy(out_sb[:], out_p[:])
            nc.scalar.dma_start(outT[:, t * VTILE:(t + 1) * VTILE], out_sb[:])

    nc.compile()
    return nc


def _get_nc():
    global _NC_CACHE
    if _NC_CACHE is None:
        _NC_CACHE = _build_kernel()
    return _NC_CACHE


def _prep_host(features, neighbor_map, neighbor_mask, kernel):
    """Build per-core device inputs.

    Returns (gts_all, wst): gts_all[c] is [NTILES, 128, NGRP*VTILE] bf16 with
    partition row (k%4)*32+ch of group k//4 holding channel ch of the voxel's
    k-th gathered neighbor; wst is the matching [128, NGRP*COUT] weight stack.
    """
    feat = np.asarray(features, dtype=np.float32)
    feat_ext = np.vstack([feat, np.zeros((1, CIN), dtype=np.float32)])
    nm = np.asarray(neighbor_map, dtype=np.int64)      # [27, N]
    mk = np.asarray(neighbor_mask, dtype=bool)          # [27, N]

    # weight stack: group g partition rows 32j..32j+31 = kernel[4g+j]
    w = np.asarray(kernel, dtype=np.float32)            # [27, 32, 64]
    wstk = np.zeros((NGRP, 4, CIN, COUT), dtype=np.float32)
    for g in range(NGRP):
        for j in range(4):
            kk = 4 * g + j
            if kk < K:
                wstk[g, j] = w[kk]
    wst = np.ascontiguousarray(
        wstk.transpose(1, 2, 0, 3).reshape(128, NGRP * COUT)
    ).astype(ml_dtypes.bfloat16)

    gts_all, gts6_all = [], []
    for c in range(NCORES):
        vloc = np.arange(NPAD)
        vglob = np.minimum(c * PERCORE + vloc, N - 1)
        valid_v = vloc < PERCORE                        # [NPAD]
        nmv = nm[:, vglob]                              # [27, NPAD]
        mskv = mk[:, vglob] & valid_v[None, :]
        src = np.where(mskv, nmv, N)                    # masked -> zero row
        g27 = feat_ext[src]                             # [27, NPAD, 32] f32
        # k 0..23: [24=(g,j), (t,col), ch] -> [t, (j, ch), g, col]
        gmain = g27[:24].reshape(6, 4, NTILES, VTILE, CIN)
        gmain = gmain.transpose(2, 1, 4, 0, 3).reshape(NTILES, 128, 6 * VTILE)
        gts_all.append(np.ascontiguousarray(gmain).astype(ml_dtypes.bfloat16))
        # k 24..26: [3=j, (tb,t7,col), ch] -> [tb, (j,ch), (t7,col)]
        g6 = g27[24:].reshape(3, NTILES // 7, 7 * VTILE, CIN)
        g6 = g6.transpose(1, 0, 3, 2).reshape(NTILES // 7, 96, 7 * VTILE)
        gts6_all.append(np.ascontiguousarray(g6).astype(ml_dtypes.bfloat16))
    return gts_all, gts6_all, wst


def _postprocess(res):
    outs = []
    for c in range(NCORES):
        oT = np.asarray(res.results[c]["outT"], dtype=np.float32)  # [64, NPAD]
        outs.append(oT.T[:PERCORE])                     # [25000, 64]
    return np.concatenate(outs, axis=0).astype(np.float32)


def kernel(features, neighbor_map, neighbor_mask, kernel):
    gts_all, gts6_all, wst = _prep_host(features, neighbor_map, neighbor_mask,
                                        kernel)
    nc = _get_nc()
    in_maps = [{"gts": gts_all[c], "gts6": gts6_all[c], "wst": wst}
               for c in range(NCORES)]
    res = run_bass_kernel_spmd(nc, in_maps, core_ids=list(range(NCORES)))
    return _postprocess(res)


# revision 14
# speedup vs baseline: 1.0274x; 1.0274x over previous
"""Sparse 3D conv (gather -> per-offset matmul -> accumulate) on 8 TRN2 NeuronCores.

Strategy (data-parallel over output voxels, per the sharding hint):
  - Shard the N=200000 output voxels across 8 cores (25000 each, padded to
    25088 = 49 tiles x 512 voxels).
  - Host-side sharding prep builds each core's operand stream in the layout
    the PE consumes directly: for every tile, a [128, 7, 512] bf16 block
    whose partition rows are (k%4)*32+ch for kernel-offset group g=k//4
    (27 offsets padded to 28 = 7 groups of 4) and whose columns are the 512
    voxels, with masked/padded entries zeroed.
  - Replicate the small [128, 7, 64] grouped weight stack to every core.
  - Device per tile: one HWDGE stream DMA (898KB), 7 PE matmuls accumulating
    out^T[64, 512] f32 over the groups in PSUM, copy to SBUF as fp16, DMA to
    a per-core out^T[64, 25088] fp16 buffer. Memory-bound: ~48MB/core at
    line rate.
  - Host transposes/crops/concats per-core outputs to the full [200000, 64].
"""
import numpy as np
import ml_dtypes
from contextlib import ExitStack

import concourse.bass as bass
import concourse.bacc as bacc
import concourse.mybir as mybir
import concourse.tile as tile
from concourse.bass_utils import run_bass_kernel_spmd

N = 200000
K = 27
CIN = 32
COUT = 64
NCORES = 8
PERCORE = N // NCORES          # 25000
VTILE = 512
NTILES = (PERCORE + VTILE - 1) // VTILE   # 49
NPAD = NTILES * VTILE          # 25088
NGRP = 7                       # 28 k-slots (27 real + 1 zero) in groups of 4

_NC_CACHE = None


def _build_kernel():
    nc = bacc.Bacc("TRN2", target_bir_lowering=False)
    gts = nc.dram_tensor("gts", [NTILES, 128, NGRP * VTILE], mybir.dt.bfloat16,
                         kind="ExternalInput")
    wst = nc.dram_tensor("wst", [128, NGRP * COUT], mybir.dt.bfloat16,
                         kind="ExternalInput")
    outT = nc.dram_tensor("outT", [COUT, NPAD], mybir.dt.float16,
                          kind="ExternalOutput")

    with tile.TileContext(nc) as tc, ExitStack() as ctx:
        const = ctx.enter_context(tc.tile_pool(name="const", bufs=1))
        sbg = ctx.enter_context(tc.tile_pool(name="sbg", bufs=6))
        sbo = ctx.enter_context(tc.tile_pool(name="sbo", bufs=4))
        opsum = ctx.enter_context(tc.tile_pool(name="opsum", bufs=4, space="PSUM"))

        w_sb = const.tile([128, NGRP * COUT], mybir.dt.bfloat16, name="w_sb")
        nc.sync.dma_start(w_sb[:], wst[:])

        for t in range(NTILES):
            gt = sbg.tile([128, NGRP * VTILE], mybir.dt.bfloat16, name="gt",
                          tag="gt")
            nc.sync.dma_start(gt[:], gts[t])

            out_p = opsum.tile([COUT, VTILE], mybir.dt.float32, name="out_p",
                               tag="op")
            for g in range(NGRP):
                nc.tensor.matmul(
                    out_p[:],
                    lhsT=w_sb[:, g * COUT:(g + 1) * COUT],
                    rhs=gt[:, g * VTILE:(g + 1) * VTILE],
                    start=(g == 0),
                    stop=(g == NGRP - 1),
                )
            out_sb = sbo.tile([COUT, VTILE], mybir.dt.float16, name="out_sb",
                              tag="ob")
            if t % 2 == 0:
                nc.vector.tensor_copy(out_sb[:], out_p[:])
            else:
                nc.scalar.copy(out_sb[:], out_p[:])
            nc.scalar.dma_start(outT[:, t * VTILE:(t + 1) * VTILE], out_sb[:])

    nc.compile()
    return nc


def _get_nc():
    global _NC_CACHE
    if _NC_CACHE is None:
        _NC_CACHE = _build_kernel()
    return _NC_CACHE


def _prep_host(features, neighbor_map, neighbor_mask, kernel):
    """Build per-core device inputs.

    Returns (gts_all, wst): gts_all[c] is [NTILES, 128, NGRP*VTILE] bf16 with
    partition row (k%4)*32+ch of group k//4 holding channel ch of the voxel's
    k-th gathered neighbor; wst is the matching [128, NGRP*COUT] weight stack.
    """
    feat = np.asarray(features, dtype=np.float32)
    feat_ext = np.vstack([feat, np.zeros((1, CIN), dtype=np.float32)])
    nm = np.asarray(neighbor_map, dtype=np.int64)      # [27, N]
    mk = np.asarray(neighbor_mask, dtype=bool)          # [27, N]

    # weight stack: group g partition rows 32j..32j+31 = kernel[4g+j]
    w = np.asarray(kernel, dtype=np.float32)            # [27, 32, 64]
    wstk = np.zeros((NGRP, 4, CIN, COUT), dtype=np.float32)
    for g in range(NGRP):
        for j in range(4):
            kk = 4 * g + j
            if kk < K:
                wstk[g, j] = w[kk]
    wst = np.ascontiguousarray(
        wstk.transpose(1, 2, 0, 3).reshape(128, NGRP * COUT)
    ).astype(ml_dtypes.bfloat16)

    gts_all = []
    for c in range(NCORES):
        vloc = np.arange(NPAD)
        vglob = np.minimum(c * PERCORE + vloc, N - 1)
        valid_v = vloc < PERCORE                        # [NPAD]
        nmv = nm[:, vglob]                              # [27, NPAD]
        mskv = mk[:, vglob] & valid_v[None, :]
        src = np.where(mskv, nmv, N)                    # masked -> zero row
        g27 = feat_ext[src]                             # [27, NPAD, 32] f32
        g28 = np.zeros((NGRP * 4, NPAD, CIN), dtype=np.float32)
        g28[:K] = g27
        # [28=(g,j), NPAD=(t,col), ch] -> [t, (j, ch), g, col]
        g28 = g28.reshape(NGRP, 4, NTILES, VTILE, CIN)
        gt = g28.transpose(2, 1, 4, 0, 3).reshape(NTILES, 128, NGRP * VTILE)
        gts_all.append(np.ascontiguousarray(gt).astype(ml_dtypes.bfloat16))
    return gts_all, wst


def _postprocess(res):
    outs = []
    for c in range(NCORES):
        oT = np.asarray(res.results[c]["outT"], dtype=np.float32)  # [64, NPAD]
        outs.append(oT.T[:PERCORE])                     # [25000, 64]
    return np.concatenate(outs, axis=0).astype(np.float32)


def kernel(features, neighbor_map, neighbor_mask, kernel):
    gts_all, wst = _prep_host(features, neighbor_map, neighbor_mask, kernel)
    nc = _get_nc()
    in_maps = [{"gts": gts_all[c], "wst": wst} for c in range(NCORES)]
    res = run_bass_kernel_spmd(nc, in_maps, core_ids=list(range(NCORES)))
    return _postprocess(res)
